# revision 93
# baseline (speedup 1.0000x reference)
"""Trainium2 Bass kernel: fused multi-head attention (N=4, L=2048, E=2048, H=16).

Sharding (8 cores): data-parallel over the 4 batches x tensor-parallel over 2
head-groups of 8 heads.  Core c handles batch c//2, head-group c%2.  Each core
computes Q/K/V projections for its head group, masked softmax attention, and
the partial output projection against its row-slice of Wo.  The two partials
per batch are summed on the host (the Wo row-parallel all-reduce) and the
output bias is added there too.

Per-core kernel layout choices (all matmuls contraction-on-partitions, no
on-device transposes -- the host ships pre-transposed activations/weights):
  - Q and K projections run in fp8(e4m3) DoubleRow perf mode (2 contraction
    chunks per pass): host pre-scales X by 16 and W by 1024 into fp8; the
    descale folds into the PSUM->SBUF copy scale.  End-to-end rel err ~1.6%
    (vs 0.40% all-bf16, tolerance 2e-2); V/scores/AV/out-proj stay bf16 --
    measured numpy sweeps put any further fp8 site over the 2% gate.
  - K^T and V live in SBUF for the whole kernel (36KB/partition): no DRAM
    spill round-trip between projection and attention.
  - scores S^T tile: [k-block=128, LH=1024] = (K^T chunk).T @ Q^T, emitted
    one chunk ahead of the AV accumulation (in-order PE queue).
  - P = exp(S^T * E^-0.5 + mask_bias[k]) on ScalarE (mask folded into the
    per-partition activation bias), output bf16.  The 9-chunk exp chain is
    the attention-phase critical path; everything else hides behind it.
  - softmax denominator: fp16 DVE accumulation (2x_1p mode) of P chunks,
    then a fp16 ones-matmul partition-reduces and broadcasts; reciprocal +
    multiply (DVE) normalizes.
  - head h+1's Q projection (2 single-bank psum groups) is emitted inside
    head h's chunk loop, so head hand-off has no serial qproj->qt->scores
    chain.
  - the V projection's (chunks 4-7, heads 4-7) groups and the first 4
    output row-blocks' projection are deferred/spliced into the attention
    loop as PE filler (attention is ScalarE-bound, PE has slack).
  - output projection: lhsT=A^T blocks, rhs=Wo^T, accumulated over heads;
    row-block stores batched into [128, E/2] DMAs.
  - DMAs are coalesced aggressively (each costs ~1.2us serialized HWDGE+SEQ
    overhead): whole-tile strided loads, ~50 DMAs total per core.
"""

from contextlib import ExitStack

import numpy as np
import ml_dtypes

P = 128          # SBUF partitions
D = 128          # head dim
G = 2            # head groups (tensor-parallel degree per batch)
NCORES = 8
BF16 = ml_dtypes.bfloat16
F8 = ml_dtypes.float8_e4m3   # device float8e4
MASK_BIAS = -60.0
# fp8 pre-scales (folded out on-device via the PSUM->SBUF copy scale)
SX = 16.0        # activations (|x| < 5.5 -> < 88, fp8e4 max 240)
SW = 1024.0      # weights (|W| < 0.12 -> < 123)
DESCALE = 1.0 / (SX * SW)

_BUILT = {}


def _build(L, E, HL, LK=None, reps=1):
    """Build the per-core Bass module (same program on every core).

    LK is the (padded) compacted key length: the host drops masked-out keys
    -- they contribute exactly zero to both the attention numerator and
    denominator -- and pads to a multiple of 128.  reps>1 repeats the whole
    computation serially inside one NEFF (scratch WAW deps order the reps)
    -- used only for slope-based HW timing."""
    if LK is None:
        LK = L
    import concourse.bass as bass
    import concourse.tile as tile
    from concourse import mybir

    bf16 = mybir.dt.bfloat16
    f16 = mybir.dt.float16
    f32 = mybir.dt.float32
    f32r = mybir.dt.float32r
    fp8 = mybir.dt.float8e4
    DR = mybir.MatmulPerfMode.DoubleRow

    EH = HL * D          # local projection width
    IC = E // P          # contraction chunks (projections)
    ICH = IC // 2        # half of the contraction chunks (split W/X loads)
    KC = LK // P         # key chunks (attention)
    ET = E // 512        # 512-wide e tiles (out proj)
    VW = min(512, EH // 2)  # v-proj dh tile width (within one W half-load)
    VT = EH // VW
    KB = 512 // P        # k blocks per 512-wide l tile (v proj)
    LH = min(1024, L)    # attention l-half width
    NLH = L // LH
    NTH = LH // 512
    SCALE = float(E) ** -0.5

    nc = bass.Bass(num_swdge_queues=4)
    xqt = nc.dram_tensor("xqt", [E, L], fp8, kind="ExternalInput")
    xkt = nc.dram_tensor("xkt", [E, LK], fp8, kind="ExternalInput")
    xvt = nc.dram_tensor("xvt", [E, LK], bf16, kind="ExternalInput")
    wqt = nc.dram_tensor("wqt", [E, EH], fp8, kind="ExternalInput")
    wkt = nc.dram_tensor("wkt", [E, EH], fp8, kind="ExternalInput")
    wvt = nc.dram_tensor("wvt", [E, EH], bf16, kind="ExternalInput")
    wot = nc.dram_tensor("wot", [EH, E], bf16, kind="ExternalInput")
    mbias = nc.dram_tensor("mbias", [P, KC], f32, kind="ExternalInput")
    out = nc.dram_tensor("out", [L, E], f32, kind="ExternalOutput")

    with tile.TileContext(nc) as tc, ExitStack() as ctx:
        # All SBUF pools are opened for the whole kernel so no SBUF address is
        # ever reused across pools (cross-pool aliasing generates WAR waits
        # against many DMA-queue processors -> "too many sync wait commands").
        singles = ctx.enter_context(tc.tile_pool(name="singles", bufs=1))
        at_pool = ctx.enter_context(tc.tile_pool(name="at", bufs=1))
        xp = ctx.enter_context(tc.tile_pool(name="xp", bufs=2))
        wp = ctx.enter_context(tc.tile_pool(name="wp", bufs=4))
        qtp = ctx.enter_context(tc.tile_pool(name="qt", bufs=2))
        ptp = ctx.enter_context(tc.tile_pool(name="ptp", bufs=3))
        dnp = ctx.enter_context(tc.tile_pool(name="dnp", bufs=2))
        oop = ctx.enter_context(tc.tile_pool(name="oo", bufs=2))

        ones_t = singles.tile([P, P], f16)
        nc.vector.memset(ones_t, 1.0)
        mb_t = singles.tile([P, KC], f32)
        # (mb_t's load is issued in body() after the K-projection loads --
        # it is only read at attention, and putting it first would add
        # ~1.3us of DMA-queue latency ahead of the opening weight load)

        at_t = at_pool.tile([P, HL, L], bf16)
        # K^T and V stay SBUF-resident (36KB/partition) -- no DRAM spill
        # round-trip between the projections and attention.
        kt_sb = singles.tile([P, HL, LK], bf16)
        v_sb = singles.tile([P, KC, EH], bf16)

        # ---- K / V projections (spilled to DRAM scratch) ----
        EHH = EH // 2  # W loaded in two dh-halves so loads pipeline (bufs=2)

        def proj(xT, wT, name, transposed_out, Lx, xdt=bf16,  # noqa: C901
                 defer_out=None):
            # When xdt is fp8, the transposed (K^T) path runs DoubleRow fp8
            # matmuls (2 contraction chunks per pass) and descales on copy.
            xv = xT.rearrange("(c p) l -> p c l", p=P)
            wv = wT.rearrange("(c p) m -> p c m", p=P)
            with tc.tile_pool(name=f"ps_{name}", bufs=6, space="PSUM") as pp:
                # half-1's load is delayed past the first X tile so the
                # first matmul group (which only needs half 0) starts sooner.
                # One strided DMA per half: every DMA pays ~1.2us of
                # serialized HWDGE+SEQ overhead, so batch aggressively.
                wts = [wp.tile([P, IC, EHH], xdt, tag="w", name=f"wt{i}")
                       for i in range(2)]

                def load_w(wh, split=False, part=None):
                    # split: first 4 chunks land first so the opening
                    # matmul group (chunks 0..3) starts ~2us sooner
                    if split or part is not None:
                        if part in (None, 0):
                            nc.sync.dma_start(
                                out=wts[wh][:, :4],
                                in_=wv[:, :4, wh * EHH:(wh + 1) * EHH])
                        if part in (None, 1):
                            nc.sync.dma_start(
                                out=wts[wh][:, 4:],
                                in_=wv[:, 4:, wh * EHH:(wh + 1) * EHH])
                    else:
                        nc.sync.dma_start(
                            out=wts[wh],
                            in_=wv[:, :, wh * EHH:(wh + 1) * EHH])



                def w_slice(j0, j1):  # dh range -> (tile, local slice)
                    wh = j0 // EHH
                    assert (j1 - 1) // EHH == wh
                    return wts[wh][:, :, j0 - wh * EHH:j1 - wh * EHH]

                tiles = []
                off = 0
                while off < Lx:
                    w = min(512, Lx - off)
                    tiles.append((off, w))
                    off += w
                # smallest tile first: shortest possible DMA before the
                # first matmul group of the phase
                tiles.sort(key=lambda t: t[1])
                for ti, (off, w) in enumerate(tiles):
                    xt = xp.tile([P, IC, 512], xdt, tag="x")
                    if ti == 0 and xdt == fp8:
                        # the opening weight quarter goes first in the DMA
                        # queue: it gates the kernel's first Ldweights
                        load_w(0, part=0)
                    if xdt == bf16 and w == 512:
                        # bf16 X tiles split into two 256-wide l-halves
                        # (elem stays 512B -- no sub-512B DMA penalty) so
                        # the first k-blocks' matmuls start a half-load
                        # earlier; fp8 l-halves would drop to 256B elems
                        # and double the transfer cost, so fp8 tiles split
                        # along the chunk dim instead (below).
                        nc.sync.dma_start(
                            out=xt[:, :, :256], in_=xv[:, :, off:off + 256])
                        nc.sync.dma_start(
                            out=xt[:, :, 256:512],
                            in_=xv[:, :, off + 256:off + 512])
                    elif xdt == fp8 and w == 512:
                        nc.sync.dma_start(
                            out=xt[:, :4, :w], in_=xv[:, :4, off:off + w])
                        nc.sync.dma_start(
                            out=xt[:, 4:, :w], in_=xv[:, 4:, off:off + w])
                    else:
                        nc.sync.dma_start(
                            out=xt[:, :, :w], in_=xv[:, :, off:off + w])
                    if ti == 0:
                        # W after the first (smallest) X tile: the opening
                        # matmul group needs X + W chunks 0-3 only
                        if xdt == fp8:
                            load_w(0, part=1)
                        else:
                            load_w(0, split=True)
                        load_w(1, split=True)
                    if transposed_out is not None:  # K^T: [d, l] per head
                        for h in range(HL):
                            ps = pp.tile([P, 512], f32, tag="ps")
                            wsl = w_slice(h * D, (h + 1) * D)
                            if xdt == fp8:
                                for c2 in range(IC // 2):
                                    nc.tensor.matmul(
                                        ps[:, :w],
                                        lhsT=wsl[:, 2 * c2:2 * c2 + 2],
                                        rhs=xt[:, 2 * c2:2 * c2 + 2, :w],
                                        start=(c2 == 0),
                                        stop=(c2 == IC // 2 - 1),
                                        perf_mode=DR)
                            else:
                                for c in range(IC):
                                    nc.tensor.matmul(
                                        ps[:, :w], lhsT=wsl[:, c],
                                        rhs=xt[:, c, :w],
                                        start=(c == 0), stop=(c == IC - 1))
                            # alternate the descale copy between the (idle)
                            # ScalarE and DVE so psum slots recycle at the
                            # fp8 matmul rate
                            if xdt == fp8:
                                if h % 2 == 0:
                                    nc.scalar.mul(
                                        out=transposed_out[:, h, off:off + w],
                                        in_=ps[:, :w], mul=DESCALE)
                                else:
                                    nc.vector.tensor_scalar_mul(
                                        transposed_out[:, h, off:off + w],
                                        ps[:, :w], DESCALE)
                            else:
                                nc.vector.tensor_copy(
                                    out=transposed_out[:, h, off:off + w],
                                    in_=ps[:, :w])
                    else:  # V: natural [k, dh]
                        for kb in range(w // P):
                            for vt_ in range(VT):
                                cb = (off + kb * P) // P
                                if (defer_out is not None and vt_ == 1
                                        and (off == 512
                                             or (off == 0 and kb == 3))):
                                    # handed to the attention phase: vt=1
                                    # chunks are only read by heads 4-7 at
                                    # their own chunk index, so heads 0-4
                                    # can emit them into the ACT-bound
                                    # loop's PE slack (host h reads come
                                    # chunks later than the emission)
                                    defer_out.append(
                                        (xt, w_slice(VW, 2 * VW), kb, cb))
                                    continue
                                ps = pp.tile([P, VW], f32, tag="ps")
                                wsl = w_slice(vt_ * VW, (vt_ + 1) * VW)
                                for c in range(IC):
                                    nc.tensor.matmul(
                                        ps, lhsT=xt[:, c, kb * P:(kb + 1) * P],
                                        rhs=wsl[:, c],
                                        start=(c == 0), stop=(c == IC - 1))
                                nc.vector.tensor_copy(
                                    out=v_sb[:, cb, vt_ * VW:(vt_ + 1) * VW],
                                    in_=ps)

        def body(rep):
          proj(xkt, wkt, f"k{rep}", kt_sb, LK, xdt=fp8)
          nc.sync.dma_start(out=mb_t, in_=mbias[:, :])
          deferred_v = []
          proj(xvt, wvt, f"v{rep}", None, LK, defer_out=deferred_v)
          # wq before wo: wq reuses the K-proj weight slots (free by now);
          # wo reuses the V-proj slots, whose vt=1 half stays live for the
          # deferred V groups until mid-attention -- wo is only needed for
          # the lh1 outproj filler, well after its (delayed) load lands.
          wq_halves = []
          for wh in range(2):  # ic-halves
              wqh = wp.tile([P, ICH, EH], fp8, tag="w")
              nc.sync.dma_start(out=wqh,
                                in_=wq_v[:, wh * ICH:(wh + 1) * ICH])
              wq_halves.append(wqh)
          wo_halves = []
          for wh in range(2 if HL > 1 else 1):
              woh = wp.tile([P, HLH, E], bf16, tag="w")
              nc.sync.dma_start(out=woh,
                                in_=wot_v[:, wh * HLH:(wh + 1) * HLH])
              wo_halves.append(woh)
          run_attention(rep, wq_halves, wo_halves, deferred_v)
          run_outproj(rep, wo_halves)

        def outproj_group(lb, eh, pp, tag, wo_halves, interleave=False):
            """One [P, E/2] output half-row-block: 2 psum groups + store."""
            f = OutprojFiller(lb, eh, pp, tag, wo_halves)
            f.finish(interleave=interleave)

        class OutprojFiller:
            """Incremental emitter for one [P, E/2] output half-block, so
            its matmuls can be spliced into PE stall slots elsewhere."""

            def __init__(self, lb, eh, pp, tag, wo_halves):
                self.lb, self.eh = lb, eh
                self.pp, self.tag = pp, tag
                self.wo = wo_halves
                self.ot = oop.tile([P, E // 2], f32, tag="o")
                self.m = 0
                self.ps = None

            def emit(self, k):
                for _ in range(k):
                    if self.m >= HL * (ET // 2):
                        return
                    g, hh = divmod(self.m, HL)
                    et = self.eh * (ET // 2) + g
                    if hh == 0:
                        self.ps = self.pp.tile([P, 512], f32, tag=self.tag)
                    nc.tensor.matmul(
                        self.ps,
                        lhsT=at_t[:, hh, self.lb * P:(self.lb + 1) * P],
                        rhs=self.wo[hh // HLH][
                            :, hh % HLH, et * 512:(et + 1) * 512],
                        start=(hh == 0), stop=(hh == HL - 1))
                    if hh == HL - 1:
                        nc.vector.tensor_copy(
                            out=self.ot[:, g * 512:(g + 1) * 512],
                            in_=self.ps)
                    self.m += 1

            def finish(self, interleave=False):
                e0 = self.eh * (E // 2)
                if interleave:
                    # store each 512-half right after its psum copy: the
                    # end-of-kernel drain then only waits one short flush
                    for g in range(ET // 2):
                        self.emit(HL * (g + 1) - self.m)
                        nc.scalar.dma_start(
                            out=out[self.lb * P:(self.lb + 1) * P,
                                    e0 + g * 512:e0 + (g + 1) * 512],
                            in_=self.ot[:, g * 512:(g + 1) * 512])
                    return
                self.emit(HL * (ET // 2) - self.m)
                nc.scalar.dma_start(
                    out=out[self.lb * P:(self.lb + 1) * P,
                            e0:e0 + E // 2],
                    in_=self.ot)

        # ---- fused Q-projection + attention ----
        # Computing Q^T per (head, l-half) right before its attention keeps
        # TensorE dense through the ScalarE-heavy softmax phase (HAM stays
        # warm) and avoids spilling Q^T to DRAM.
        xq_v = xqt.rearrange("(c p) l -> p c l", p=P)
        wq_v = wqt.rearrange("(c p) m -> p c m", p=P)

        def run_attention(rep, wq_halves, wo_halves, deferred_v):
          # qpp opens first: its banks alias the projection-phase psum
          # slots that freed earliest, so the first q-projection groups
          # don't wait on the tail of the V projection.
          with tc.tile_pool(name=f"qps{rep}", bufs=2, space="PSUM") as qpp, \
               tc.tile_pool(name=f"stps{rep}", bufs=2, space="PSUM") as stp, \
               tc.tile_pool(name=f"otps{rep}", bufs=1, space="PSUM") as otp:

            class VFill:
                """Deferred V-projection group (16 matmuls + copy), spliced
                into lh0's ACT-bound loop 4 matmuls per chunk."""

                def __init__(self, xt, wsl, kb, cb):
                    self.xt, self.wsl = xt, wsl
                    self.kb, self.cb = kb, cb
                    self.m = 0
                    self.ps = None

                def emit(self, k):
                    for _ in range(k):
                        if self.m >= IC:
                            return
                        if self.m == 0:
                            self.ps = qpp.tile([P, VW], f32, tag="q",
                                               name=f"vf{self.cb}")
                        c = self.m
                        nc.tensor.matmul(
                            self.ps,
                            lhsT=self.xt[:, c,
                                         self.kb * P:(self.kb + 1) * P],
                            rhs=self.wsl[:, c],
                            start=(c == 0), stop=(c == IC - 1))
                        self.m += 1
                        if self.m == IC:
                            nc.vector.tensor_copy(
                                out=v_sb[:, self.cb, VW:2 * VW],
                                in_=self.ps)

            def load_xq(lhx):
                l0x = lhx * LH
                halves = []
                for wh in range(2):  # ic-halves of this l-half of X_q^T
                    xqh = xp.tile([P, ICH, LH], fp8, tag="x2",
                                  name=f"xq{lhx}_{wh}")
                    nc.sync.dma_start(
                        out=xqh,
                        in_=xq_v[:, wh * ICH:(wh + 1) * ICH, l0x:l0x + LH])
                    halves.append(xqh)
                return halves

            xqs = {0: load_xq(0)}
            qt_first = {}
            st_stash = None  # next head's pre-scored c0 tile
            for lh in range(NLH):
                l0 = lh * LH
                xq_halves = xqs[lh]

                def qproj_nt(hq, nt, qt_dst, xqh=None):
                    """One [P,512] Q^T psum group (fp8 DoubleRow over 8
                    chunk pairs) + its descaled copy into qt_dst. Split
                    per-nt so q double-buffers in 2 single-bank psum slots
                    -- lets head h+1's Q be built during head h. The copy
                    stays on ScalarE: DVE placements measurably lose more
                    to its in-order queue than they save."""
                    if xqh is None:
                        xqh = xq_halves
                    q_ps = qpp.tile([P, 512], f32, tag="q")
                    for c2 in range(IC // 2):
                        wh = (2 * c2) // ICH
                        lc = 2 * c2 - wh * ICH
                        nc.tensor.matmul(
                            q_ps,
                            lhsT=wq_halves[wh][
                                :, lc:lc + 2, hq * D:(hq + 1) * D],
                            rhs=xqh[wh][
                                :, lc:lc + 2, nt * 512:(nt + 1) * 512],
                            start=(c2 == 0), stop=(c2 == IC // 2 - 1),
                            perf_mode=DR)
                    nc.scalar.mul(out=qt_dst[:, nt * 512:(nt + 1) * 512],
                                  in_=q_ps, mul=DESCALE)

                if lh in qt_first:
                    qt_t = qt_first[lh]  # built during the previous l-half
                else:
                    qt_t = qtp.tile([P, LH], bf16, tag="qt")
                    for nt in range(NTH):
                        qproj_nt(0, nt, qt_t)
                for h in range(HL):
                    cross = (h + 1 == HL and lh + 1 < NLH)
                    if cross:
                        # prefetch the next l-half's X_q now (slots' WAR
                        # cleared -- this l-half's last Q groups were
                        # emitted during head h-1) and build (lh+1, h0)'s
                        # Q during this head, like any other hand-off
                        xqs[lh + 1] = load_xq(lh + 1)
                    if h + 1 < HL or cross:
                        qt_next = qtp.tile([P, LH], bf16, tag="qt",
                                           name=f"qtn{lh}_{h}")
                        nxt_h = (h + 1) % HL
                        nxt_xq = xqs[lh + 1] if cross else None
                        if cross:
                            qt_first[lh + 1] = qt_next
                    else:
                        qt_next = None
                    ot_ps = otp.tile([P, LH], f32, tag="ot")
                    # fp16 den: 2-byte operands turn on the DVE 2x_1p mode
                    # for the chunk accumulation (f16 keeps ~11 mantissa
                    # bits -- den ~2.5k so rounding stays ~1e-4 relative)
                    den = dnp.tile([P, LH], f16, tag="den")

                    # scores run one chunk ahead of the AV accumulation so
                    # the in-order PE queue never head-of-line blocks on the
                    # exp of the chunk it just scored
                    def scores(c, hs=None, qts=None):
                        hs = h if hs is None else hs
                        qts = qt_t if qts is None else qts
                        st = stp.tile([P, LH], f32, tag="st")
                        for nt in range(NTH):
                            nc.tensor.matmul(
                                st[:, nt * 512:(nt + 1) * 512],
                                lhsT=kt_sb[:, hs, c * P:(c + 1) * P],
                                rhs=qts[:, nt * 512:(nt + 1) * 512],
                                start=True, stop=True)
                        return st

                    def av(c, pt):
                        if c == 0:
                            nc.vector.tensor_copy(out=den, in_=pt)
                        else:
                            nc.vector.tensor_add(out=den, in0=den, in1=pt)
                        for nt in range(NTH):
                            nc.tensor.matmul(
                                ot_ps[:, nt * 512:(nt + 1) * 512],
                                lhsT=v_sb[:, c, h * D:(h + 1) * D],
                                rhs=pt[:, nt * 512:(nt + 1) * 512],
                                start=(c == 0), stop=(c == KC - 1))

                    # During the second l-half, output-projection matmuls for
                    # the finished first l-half are spliced into the chunk
                    # loop (2 per chunk): they soak up the ~400ns/chunk PE
                    # stall in the ACT-bound exp chain.
                    filler = (OutprojFiller(h // 2, h % 2, qpp, "q",
                                            wo_halves)
                              if lh == 1 else None)
                    vfill = (VFill(*deferred_v[h])
                             if lh == 0 and h < len(deferred_v) else None)
                    prev_pt = None
                    for c in range(KC):
                        if c == 0 and st_stash is not None:
                            st = st_stash  # scored during the previous head
                        else:
                            st = scores(c)
                        if prev_pt is not None:
                            av(c - 1, prev_pt)
                            if filler is not None:
                                filler.emit(2)
                            if vfill is not None:
                                vfill.emit(4)
                        # build the NEXT head's Q mid-loop: by this head's
                        # end its qt is ready, so the head hand-off has no
                        # serial qproj->qt->scores->exp chain
                        if qt_next is not None and c in (4, 6):
                            qproj_nt(nxt_h, int(c == 6), qt_next,
                                     xqh=nxt_xq)
                        pt = ptp.tile([P, LH], bf16, tag="pt")
                        nc.scalar.activation(
                            out=pt, in_=st,
                            func=mybir.ActivationFunctionType.Exp,
                            bias=mb_t[:, c:c + 1], scale=SCALE)
                        prev_pt = pt
                    # early first-scores for the next head: PE fills its
                    # wait on exp(c8) with useful work and the next head's
                    # exp chain starts right after exp(c8) (the per-head
                    # exp cadence is the lh0 critical path)
                    st_stash = (scores(0, hs=nxt_h, qts=qt_next)
                                if qt_next is not None else None)
                    if lh == NLH - 1 and h == HL - 1:
                        # last-head hand-off to the outproj tail: emit the
                        # first tail groups here (from the q psum slots) --
                        # they fill this head's exp(c8) wait and dodge the
                        # fresh psum pool's WAR on the attention banks
                        outproj_group(HL // 2, 0, qpp, "q", wo_halves)
                        outproj_group(HL // 2, 1, qpp, "q", wo_halves)
                    av(KC - 1, prev_pt)
                    # denominator: partition-reduce + broadcast via ones-matmul
                    bc = stp.tile([P, LH], f32, tag="st")
                    for nt in range(NTH):
                        nc.tensor.matmul(
                            bc[:, nt * 512:(nt + 1) * 512],
                            lhsT=ones_t[:, :],
                            rhs=den[:, nt * 512:(nt + 1) * 512],
                            start=True, stop=True)
                    rec = dnp.tile([P, LH], f32, tag="den")
                    nc.vector.reciprocal(out=rec, in_=bc)
                    nc.vector.tensor_mul(out=at_t[:, h, l0:l0 + LH],
                                         in0=ot_ps, in1=rec)
                    if filler is not None:
                        filler.finish()
                    qt_t = qt_next

        # ---- output projection ----
        wot_v = wot.rearrange("(h p) e -> p h e", p=P)
        HLH = max(1, HL // 2)

        def run_outproj(rep, wo_halves):
          # First 4 row-blocks were emitted as attention-phase filler.
          with tc.tile_pool(name=f"ops{rep}", bufs=4, space="PSUM") as opp:
            for lb in range(HL // 2, L // P):
                for eh in range(2):
                    if lb == HL // 2:
                        continue  # emitted inside the last attention head
                    last = (lb == L // P - 1 and eh == 1)
                    outproj_group(lb, eh, opp, "ps", wo_halves,
                                  interleave=last)

        for rep in range(reps):
            body(rep)

    # Split multi-wait sync_infos (TRN2 instructions carry at most one wait;
    # only the Bacc path runs this pass by default).
    import bass_rust
    bass_rust.move_matmul_waits_to_ldweights(nc.m)
    bass_rust.generate_event_semaphores(nc)
    return nc


def _get_nc(L, E, HL, LK=None):
    key = (L, E, HL, LK)
    if key not in _BUILT:
        _BUILT[key] = _build(L, E, HL, LK=LK)
    return _BUILT[key]


def _core_inputs(query_n, kc_n, vc_n, mb_n, Wq, Wk, Wv, Wo, g, HL, LK):
    """Host-side shard prep for one core: transpose + bf16-cast the batch's
    (key-compacted) activations and the head-group's weight slices."""
    EH = HL * D
    sl = slice(g * EH, (g + 1) * EH)
    return {
        "xqt": np.ascontiguousarray((query_n.T * SX).astype(F8)),
        "xkt": np.ascontiguousarray((kc_n.T * SX).astype(F8)),
        "xvt": vc_n.T.astype(BF16, order="C"),
        "wqt": np.ascontiguousarray((Wq[sl, :].T * SW).astype(F8)),
        "wkt": np.ascontiguousarray((Wk[sl, :].T * SW).astype(F8)),
        "wvt": Wv[sl, :].T.astype(BF16, order="C"),
        "wot": Wo[:, sl].T.astype(BF16, order="C"),
        "mbias": np.ascontiguousarray(mb_n.reshape(LK // P, P).T,
                                      dtype=np.float32),
    }


def _shard_inputs(query, keys, values, mask, Wq, Wk, Wv, Wo):
    """Build the 8 per-core input maps.

    Masked-out keys are dropped entirely (they contribute exactly zero to
    both the softmax numerator and denominator), and key/value sequences are
    zero-padded to a common length LK (multiple of 128); the pad positions
    are suppressed through the exp mask-bias.
    """
    N, L, E = query.shape
    HL = 16 // G
    nks = [int(mask[n].sum()) for n in range(N)]
    LK = max(P, -(-max(nks) // P) * P)
    LK = min(LK, L)

    per_batch = []
    for n in range(N):
        if LK == L and nks[n] == L:
            kc, vc = keys[n], values[n]
            mb = np.zeros(L, np.float32)
        else:
            idx = np.flatnonzero(mask[n] != 0)[:LK]
            kc = np.zeros((LK, E), np.float32)
            vc = np.zeros((LK, E), np.float32)
            kc[:idx.size] = keys[n][idx]
            vc[:idx.size] = values[n][idx]
            mb = np.full(LK, MASK_BIAS, np.float32)
            mb[:idx.size] = 0.0
        per_batch.append((kc, vc, mb))

    in_maps = []
    for c in range(NCORES):
        n, g = divmod(c, G)
        kc, vc, mb = per_batch[n]
        in_maps.append(_core_inputs(
            query[n], kc, vc, mb, Wq, Wk, Wv, Wo, g, HL, LK))
    return in_maps, L, E, HL, LK


def kernel(query, keys, values, mask, Wq, Wk, Wv, Wo, bo):
    from concourse.bass_utils import run_bass_kernel_spmd

    query = np.asarray(query, dtype=np.float32)
    keys = np.asarray(keys, dtype=np.float32)
    values = np.asarray(values, dtype=np.float32)
    mask = np.asarray(mask)
    Wq = np.asarray(Wq, dtype=np.float32)
    Wk = np.asarray(Wk, dtype=np.float32)
    Wv = np.asarray(Wv, dtype=np.float32)
    Wo = np.asarray(Wo, dtype=np.float32)
    bo = np.asarray(bo, dtype=np.float32)

    in_maps, L, E, HL, LK = _shard_inputs(
        query, keys, values, mask, Wq, Wk, Wv, Wo)
    nc = _get_nc(L, E, HL, LK)

    res = run_bass_kernel_spmd(nc, in_maps, core_ids=list(range(NCORES)))

    N = query.shape[0]
    out = np.empty((N, L, E), np.float32)
    for n in range(N):
        acc = res.results[G * n]["out"].copy()
        for g in range(1, G):
            acc += res.results[G * n + g]["out"]
        out[n] = acc + bo[None, :]
    return out

